# revision 1
# baseline (speedup 1.0000x reference)
"""Trainium2 Bass kernel for the LeViT-style attention block (16x1024x512,
16 heads, relative-position bias), data-parallel over batch on 8 NeuronCores.

kernel(**inputs) takes the full unsharded inputs and returns the full output.
"""
import numpy as np
import ml_dtypes
from contextlib import ExitStack

import concourse.bass as bass
import concourse.bacc as bacc
import concourse.tile as tile
from concourse import mybir
from concourse.dve_ops import RECIPROCAL_APPROX_FAST, RECIP_APPROX_FAST_CONSTS
from concourse.alu_op_type import AluOpType as ALU

F32 = mybir.dt.float32
BF16 = mybir.dt.bfloat16
AF = mybir.ActivationFunctionType

DIM = 512
KD = 32
HEADS = 16
VD = 128
NTOK = 1024
SCALE = KD ** -0.5
EPS = 1e-5
B_LOC = 2
TOK = B_LOC * NTOK
OUT_DIM = HEADS * VD
N_CORES = 8


def host_prep(inputs):
    """Full inputs -> (per-core in_maps list, flags dict)."""
    x = np.asarray(inputs["x"], np.float32)
    gamma = np.asarray(inputs["gamma"], np.float32)
    beta = np.asarray(inputs["beta"], np.float32)
    Wqkv = np.asarray(inputs["Wqkv"], np.float32)
    bqkv = np.asarray(inputs["bqkv"], np.float32)
    Wproj = np.asarray(inputs["Wproj"], np.float32)
    bproj = np.asarray(inputs["bproj"], np.float32)
    ab = np.asarray(inputs["attention_biases"], np.float32)
    bidx = np.asarray(inputs["bias_idxs"])

    W3 = Wqkv.reshape(HEADS, 2 * KD + VD, DIM)
    b3 = bqkv.reshape(HEADS, 2 * KD + VD)
    Wq, Wk, Wv = W3[:, :KD, :], W3[:, KD:2 * KD, :], W3[:, 2 * KD:, :]
    bq = b3[:, :KD].reshape(-1)
    bk = b3[:, KD:2 * KD].reshape(-1)
    bv = b3[:, 2 * KD:].reshape(-1)

    wqk = np.concatenate([Wq.reshape(-1, DIM), Wk.reshape(-1, DIM)], 0)
    wqkT = np.ascontiguousarray(wqk.T)
    wvT = np.ascontiguousarray(Wv.reshape(-1, DIM).T)
    wprojT = np.ascontiguousarray(Wproj.T)
    bqk = np.concatenate([bq, bk])
    bqk_t = np.ascontiguousarray(bqk.reshape(8, 128).T)
    gamma_t = np.ascontiguousarray(gamma.reshape(4, 128).T)
    beta_t = np.ascontiguousarray(beta.reshape(4, 128).T)

    bf = ml_dtypes.bfloat16
    expb = np.exp(ab[:, bidx]).astype(bf)      # [16, 1024, 1024]

    flags = {
        "HAS_BQK": bool(np.any(bqk != 0)),
        "HAS_BV": bool(np.any(bv != 0)),
        "HAS_BPROJ": bool(np.any(bproj != 0)),
        "HAS_GB": bool(np.any(gamma != 1) or np.any(beta != 0)),
    }
    shared = {
        "wqkT": wqkT.astype(bf),
        "wvT": wvT.astype(bf),
        "wprojT": wprojT.astype(bf),
        "bqk": bqk_t,
        "bv": bv.reshape(1, -1).astype(bf),
        "bproj": bproj.reshape(1, -1).astype(bf),
        "gamma": gamma_t,
        "beta": beta_t,
        "ident": np.eye(128, dtype=bf),
        "ones": np.ones((128, 128), dtype=bf),
        "expb": expb,
    }
    in_maps = []
    for c in range(N_CORES):
        m = dict(shared)
        m["x"] = np.ascontiguousarray(x[2 * c:2 * c + 2].reshape(TOK, DIM))
        in_maps.append(m)
    return in_maps, flags


def assemble(results):
    out = np.empty((16, NTOK, DIM), np.float32)
    for c in range(N_CORES):
        out[2 * c:2 * c + 2] = results[c]["y"].reshape(B_LOC, NTOK, DIM)
    return out


def build_nc(flags, loop_n=1):
    nc = bacc.Bacc("TRN2", target_bir_lowering=False, debug=False)
    dt = nc.dram_tensor
    x = dt("x", [TOK, DIM], F32, kind="ExternalInput").ap()
    wqkT = dt("wqkT", [DIM, 1024], BF16, kind="ExternalInput").ap()
    wvT = dt("wvT", [DIM, OUT_DIM], BF16, kind="ExternalInput").ap()
    wprojT = dt("wprojT", [OUT_DIM, DIM], BF16, kind="ExternalInput").ap()
    bqk = dt("bqk", [128, 8], F32, kind="ExternalInput").ap()
    bv = dt("bv", [1, OUT_DIM], BF16, kind="ExternalInput").ap()
    bproj = dt("bproj", [1, DIM], BF16, kind="ExternalInput").ap()
    gamma = dt("gamma", [128, 4], F32, kind="ExternalInput").ap()
    beta = dt("beta", [128, 4], F32, kind="ExternalInput").ap()
    ident = dt("ident", [128, 128], BF16, kind="ExternalInput").ap()
    ones = dt("ones", [128, 128], BF16, kind="ExternalInput").ap()
    expb = dt("expb", [HEADS, NTOK, NTOK], BF16, kind="ExternalInput").ap()
    otT = dt("otT", [B_LOC, HEADS, VD, NTOK], BF16, kind="Internal").ap()
    y = dt("y", [TOK, DIM], F32, kind="ExternalOutput").ap()

    HAS_BQK = flags["HAS_BQK"]
    HAS_BV = flags["HAS_BV"]
    HAS_BPROJ = flags["HAS_BPROJ"]
    HAS_GB = flags["HAS_GB"]

    with tile.TileContext(nc) as tc:
        with ExitStack() as octx:
            oep = octx.enter_context
            consts = oep(tc.tile_pool(name="consts", bufs=1))
            qkT_p = oep(tc.tile_pool(name="qkT", bufs=1))
            v_p = oep(tc.tile_pool(name="vall", bufs=1))
            wpr_p = oep(tc.tile_pool(name="wpr", bufs=1))

            ident_sb = consts.tile([128, 128], BF16)
            nc.sync.dma_start(out=ident_sb, in_=ident)
            ones_sb = consts.tile([128, 128], BF16)
            nc.sync.dma_start(out=ones_sb, in_=ones)
            eps_sb = consts.tile([128, 1], F32)
            nc.vector.memset(eps_sb, EPS)
            bqk_sb = consts.tile([128, 8], F32)
            nc.sync.dma_start(out=bqk_sb, in_=bqk)
            bv_sb = consts.tile([1, OUT_DIM], BF16)
            nc.sync.dma_start(out=bv_sb, in_=bv)
            bpr_sb = consts.tile([1, DIM], BF16)
            nc.sync.dma_start(out=bpr_sb, in_=bproj)
            gam_sb = consts.tile([128, 4], F32)
            nc.sync.dma_start(out=gam_sb, in_=gamma)
            bet_sb = consts.tile([128, 4], F32)
            nc.sync.dma_start(out=bet_sb, in_=beta)

            qkT = [qkT_p.tile([128, TOK], BF16, name=f"qkT{mt}")
                   for mt in range(8)]
            v_sb = [v_p.tile([128, 1024], BF16, name=f"v{i}")
                    for i in range(32)]

            def vtile(tt, h):
                return v_sb[tt * 2 + h // 8][:, (h % 8) * 128:(h % 8 + 1) * 128]

            loop_ctx = tc.For_i(0, loop_n, 1) if loop_n > 1 else None
            if loop_ctx is not None:
                loop_ctx.__enter__()

            # ================= Phase A+B =================
            with ExitStack() as ctx:
                ep = ctx.enter_context
                xnT_p = ep(tc.tile_pool(name="xnT", bufs=1))
                wqk_p = ep(tc.tile_pool(name="wqk", bufs=1))
                wv_p = ep(tc.tile_pool(name="wv", bufs=1))
                xt_p = ep(tc.tile_pool(name="xt", bufs=3))
                xh_p = ep(tc.tile_pool(name="xh", bufs=3))
                st_p = ep(tc.tile_pool(name="st", bufs=4))
                psAB = ep(tc.tile_pool(name="psAB", bufs=3, space="PSUM"))

                wqk_sb = []
                wv_sb = []

                def load_w():
                    for kt in range(4):
                        t = wv_p.tile([128, OUT_DIM], BF16, name=f"wv{kt}")
                        nc.sync.dma_start(out=t, in_=wvT[kt * 128:(kt + 1) * 128, :])
                        wv_sb.append(t)
                    for kt in range(4):
                        t = wqk_p.tile([128, 1024], BF16, name=f"wqk{kt}")
                        nc.sync.dma_start(out=t, in_=wqkT[kt * 128:(kt + 1) * 128, :])
                        wqk_sb.append(t)

                xnT = [xnT_p.tile([128, TOK], BF16, name=f"xnT{kt}")
                       for kt in range(4)]

                def emit_qkT(nch):
                    for mt in range(8):
                        ps = psAB.tile([128, 512], F32, tag="psB", bufs=2)
                        for kt in range(4):
                            nc.tensor.matmul(
                                ps,
                                lhsT=wqk_sb[kt][:, mt * 128:(mt + 1) * 128],
                                rhs=xnT[kt][:, nch * 512:(nch + 1) * 512],
                                start=(kt == 0), stop=(kt == 3))
                        dst = qkT[mt][:, nch * 512:(nch + 1) * 512]
                        if HAS_BQK:
                            nc.vector.tensor_scalar_add(
                                out=dst, in0=ps, scalar1=bqk_sb[:, mt:mt + 1])
                        else:
                            nc.any.tensor_copy(out=dst, in_=ps)

                def emit_v(tt):
                    for half in range(2):
                        ps = psAB.tile([128, 1024], F32, tag="psAB")
                        for sub in range(2):
                            vsl = slice(half * 1024 + sub * 512,
                                        half * 1024 + (sub + 1) * 512)
                            psl = slice(sub * 512, (sub + 1) * 512)
                            for kt in range(4):
                                nc.tensor.matmul(
                                    ps[:, psl],
                                    lhsT=xnT[kt][:, tt * 128:(tt + 1) * 128],
                                    rhs=wv_sb[kt][:, vsl],
                                    start=(kt == 0),
                                    stop=(kt == 3 and not HAS_BV))
                            if HAS_BV:
                                nc.tensor.matmul(
                                    ps[:, psl], lhsT=ones_sb[0:1, :],
                                    rhs=bv_sb[0:1, vsl],
                                    start=False, stop=True)
                        nc.any.tensor_copy(out=v_sb[tt * 2 + half], in_=ps)

                for tt in range(16):
                    xt = xt_p.tile([128, DIM], F32, tag="xt")
                    nc.sync.dma_start(out=xt, in_=x[tt * 128:(tt + 1) * 128, :])
                    if tt == 0:
                        load_w()
                    stats = st_p.tile([128, 6], F32, tag="stats")
                    nc.vector.bn_stats(out=stats, in_=xt)
                    mv = st_p.tile([128, 2], F32, tag="mv")
                    nc.vector.bn_aggr(out=mv, in_=stats)
                    sq = st_p.tile([128, 1], F32, tag="sq")
                    nc.scalar.activation(out=sq, in_=mv[:, 1:2], func=AF.Sqrt,
                                         bias=eps_sb)
                    rstd = st_p.tile([128, 1], F32, tag="rstd")
                    nc.vector.reciprocal(out=rstd, in_=sq)
                    xh = xh_p.tile([128, DIM], BF16, tag="xh")
                    nc.vector.tensor_scalar(out=xh, in0=xt, scalar1=mv[:, 0:1],
                                            scalar2=rstd, op0=ALU.subtract,
                                            op1=ALU.mult)
                    for kt in range(4):
                        pt = psAB.tile([128, 2048], BF16, tag="psAB")
                        nc.tensor.transpose(pt[:, 0:128],
                                            xh[:, kt * 128:(kt + 1) * 128],
                                            ident_sb)
                        dst = xnT[kt][:, tt * 128:(tt + 1) * 128]
                        if HAS_GB:
                            nc.vector.tensor_scalar(
                                out=dst, in0=pt[:, 0:128],
                                scalar1=gam_sb[:, kt:kt + 1],
                                scalar2=bet_sb[:, kt:kt + 1],
                                op0=ALU.mult, op1=ALU.add)
                        else:
                            nc.any.tensor_copy(out=dst, in_=pt[:, 0:128])
                    emit_v(tt)
                    if tt % 4 == 3:
                        emit_qkT(tt // 4)

            # ================= Phase C: attention =================
            with ExitStack() as ctx:
                ep = ctx.enter_context
                eb_p = ep(tc.tile_pool(name="eb", bufs=12))
                es_p = ep(tc.tile_pool(name="es", bufs=4))
                prod_p = ep(tc.tile_pool(name="prod", bufs=4))
                ot_p = ep(tc.tile_pool(name="ot", bufs=3))
                rb_p = ep(tc.tile_pool(name="rb", bufs=2))
                rc_p = ep(tc.tile_pool(name="rc", bufs=2))
                psS = ep(tc.tile_pool(name="psS", bufs=2, space="PSUM"))
                psO = ep(tc.tile_pool(name="psO", bufs=2, space="PSUM"))
                psR = ep(tc.tile_pool(name="psR", bufs=1, space="PSUM"))

                for h in range(HEADS):
                    qg = qkT[h // 4]
                    kg = qkT[4 + h // 4]
                    ro = 32 * (h % 4)
                    ebs = []
                    for mt in range(8):
                        eb = eb_p.tile([128, 1024], BF16, tag="eb")
                        nc.sync.dma_start(
                            out=eb,
                            in_=expb[h, mt * 128:(mt + 1) * 128, :])
                        ebs.append(eb)
                    for b in range(B_LOC):
                        bsl = slice(b * 1024, (b + 1) * 1024)
                        otp0 = psO.tile([128, 512], F32, tag="psO")
                        otp1 = psO.tile([128, 512], F32, tag="psO")
                        rsp = psR.tile([128, 1024], F32, tag="psR")
                        for mt in range(8):
                            msl = slice(b * 1024 + mt * 128,
                                        b * 1024 + (mt + 1) * 128)
                            ssp = psS.tile([128, 1024], F32, tag="psS")
                            for half in range(2):
                                nsl = slice(b * 1024 + half * 512,
                                            b * 1024 + (half + 1) * 512)
                                nc.tensor.matmul(
                                    ssp[:, half * 512:(half + 1) * 512],
                                    lhsT=kg[ro:ro + 32, msl],
                                    rhs=qg[ro:ro + 32, nsl],
                                    start=True, stop=True,
                                    tile_position=(ro, 0))
                            es = es_p.tile([128, 1024], BF16, tag="es")
                            nc.scalar.activation(out=es, in_=ssp, func=AF.Exp,
                                                 scale=float(SCALE))
                            prod = prod_p.tile([128, 1024], BF16, tag="prod")
                            nc.vector.tensor_tensor(out=prod, in0=es,
                                                    in1=ebs[mt], op=ALU.mult)
                            vt = vtile(b * 8 + mt, h)
                            for half, otp in ((0, otp0), (1, otp1)):
                                hsl = slice(half * 512, (half + 1) * 512)
                                nc.tensor.matmul(
                                    otp, lhsT=vt, rhs=prod[:, hsl],
                                    start=(mt == 0), stop=(mt == 7))
                            nc.tensor.matmul(
                                rsp[0:1, 0:512], lhsT=ones_sb[:, 0:1],
                                rhs=prod[:, 0:512],
                                start=(mt == 0), stop=(mt == 7))
                            nc.tensor.matmul(
                                rsp[0:1, 512:1024], lhsT=ones_sb[:, 0:1],
                                rhs=prod[:, 512:1024],
                                start=(mt == 0), stop=(mt == 7))
                        rc = rc_p.tile([33, 1024], F32, tag="rc")
                        nc.vector._custom_dve(
                            RECIPROCAL_APPROX_FAST,
                            out=rc[0:1, 0:1024], in0=rsp[0:1, 0:1024],
                            s0=RECIP_APPROX_FAST_CONSTS["s0"],
                            s1=RECIP_APPROX_FAST_CONSTS["s1"],
                            imm2=RECIP_APPROX_FAST_CONSTS["imm2"])
                        rb = rb_p.tile([128, 1024], F32, tag="rb")
                        nc.gpsimd.partition_broadcast(rb[:, 0:512],
                                                      rc[0:1, 0:512])
                        nc.gpsimd.partition_broadcast(rb[:, 512:1024],
                                                      rc[0:1, 512:1024])
                        ot = ot_p.tile([128, 1024], BF16, tag="ot")
                        nc.vector.tensor_tensor(out=ot[:, 0:512], in0=otp0,
                                                in1=rb[:, 0:512], op=ALU.mult)
                        nc.vector.tensor_tensor(out=ot[:, 512:1024], in0=otp1,
                                                in1=rb[:, 512:1024],
                                                op=ALU.mult)
                        nc.sync.dma_start(out=otT[b, h, :, :], in_=ot)

            # ================= Phase D: proj =================
            with ExitStack() as ctx:
                ep = ctx.enter_context
                og_p = ep(tc.tile_pool(name="og", bufs=3))
                yt_p = ep(tc.tile_pool(name="yt", bufs=3))
                psD = ep(tc.tile_pool(name="psD", bufs=3, space="PSUM"))

                wpr_sb = []
                for h in range(HEADS):
                    t = wpr_p.tile([128, DIM], BF16, name=f"wpr{h}")
                    nc.sync.dma_start(out=t, in_=wprojT[h * 128:(h + 1) * 128, :])
                    wpr_sb.append(t)

                for b in range(B_LOC):
                    for nt in range(8):
                        og = og_p.tile([128, 16, 128], BF16, tag="og")
                        nc.sync.dma_start(
                            out=og,
                            in_=otT[b, :, :, nt * 128:(nt + 1) * 128]
                                .rearrange("h v n -> v h n"))
                        ps = psD.tile([128, 512], F32, tag="psD")
                        for h in range(HEADS):
                            nc.tensor.matmul(
                                ps, lhsT=og[:, h, :],
                                rhs=wpr_sb[h],
                                start=(h == 0),
                                stop=(h == HEADS - 1 and not HAS_BPROJ))
                        if HAS_BPROJ:
                            nc.tensor.matmul(
                                ps, lhsT=ones_sb[0:1, :], rhs=bpr_sb[0:1, :],
                                start=False, stop=True)
                        yt = yt_p.tile([128, 512], F32, tag="yt")
                        nc.any.tensor_copy(out=yt, in_=ps)
                        nc.sync.dma_start(
                            out=y[(b * 8 + nt) * 128:(b * 8 + nt + 1) * 128, :],
                            in_=yt)

            if loop_ctx is not None:
                loop_ctx.__exit__(None, None, None)

    nc.compile()
    return nc


def kernel(**inputs):
    """Full-input entry point: shard over 8 cores, run Bass kernel, gather."""
    from concourse.bass_utils import run_bass_kernel_spmd
    in_maps, flags = host_prep(inputs)
    nc = build_nc(flags)
    res = run_bass_kernel_spmd(nc, in_maps, core_ids=list(range(N_CORES)))
    return assemble(res.results)

